# revision 36
# baseline (speedup 1.0000x reference)
"""Trainium2 Bass kernel for nn_DecoupleModel (GNN message passing), 8 NeuronCores.

Strategy v2 (graph/data parallel over nodes, fp8 DoubleRow scatter):
 - 10000 nodes sharded 8 ways (1250/core, padded to 1280 = 10 windows of 128).
 - Replicated layer-1 front: every core computes m1 = relu(x W0 + b0) and
   z1 = m1 W1 for ALL 10240 padded nodes in bf16, quantizing z1*dinv to
   fp8e4m3 (per-layer scale c_l keeps values well under the 240 max) straight
   into the node-major message buffer zsb.  No AllGather for layer 1, which
   hides the ~50us first-collective warmup entirely.
 - Scatter-add as dense matmul fT = z.T @ A with BOTH operands fp8e4m3 and
   perf_mode=DoubleRow (2 fp8 MACs/cell/cycle): A blocks hold exact {0,1,2}
   one-hot entries, z is the quantized message.  Contraction pairs two
   128-source windows per virtual tile (40 tiles), halving PE column-streams
   vs bf16.  A (13.1MB fp8) stays fully resident in SBUF.
 - Dest-half pipelining: each A pass accumulates dest windows 0-5 first
   (psum banks), runs the relu/dense/quantize epilogue for that half and
   fires its AllGather while the PE continues on dest windows 6-9; the
   second half's AllGather overlaps the next pass's first-source-half
   matmuls.  Collectives carry fp8 (half the bytes of bf16).
 - m2/m3 kept feat-major in bf16; z_{l+1} produced node-major directly via
   m-block.T @ W matmuls (identity for layer 3), no PE transposes.
 - FC head feat-major in bf16, alpha and biases folded host-side.
 - Host computes per-layer fp8 scale factors from a cheap exact CPU forward.
"""

import sys

sys.path.insert(0, "/opt/trn_rl_repo")

import numpy as np
import ml_dtypes

import concourse.bacc as bacc
import concourse.bass as bass
import concourse.mybir as mybir
import concourse.tile as tile
from concourse.bass_utils import run_bass_kernel_spmd

N_CORES = 8
N = 10000
E = 320000
IN = 256
MP = 256
FL = 512
OUT = 64

NSH_REAL = N // N_CORES          # 1250 real nodes per core
NSH = 1280                       # padded shard width (10 windows of 128)
NW = NSH // 128                  # 10 windows per shard
NFULL = NSH * N_CORES            # 10240 padded global nodes
NK = NFULL // 128                # 80 source windows
NVT = NK // 2                    # 40 DoubleRow virtual tiles
NXCH = NFULL // 512              # 20 front chunks of 512 nodes

F32 = mybir.dt.float32
F32R = mybir.dt.float32r
BF16 = mybir.dt.bfloat16
FP8 = mybir.dt.float8e4

RELU = mybir.ActivationFunctionType.Relu
COPY = mybir.ActivationFunctionType.Copy
DR = mybir.MatmulPerfMode.DoubleRow
MULT = mybir.AluOpType.mult
ADD = mybir.AluOpType.add

SCALE_TGT = 120.0                # fp8 target max (TRN e4m3 tops out at 240)

# AllGather halves by window: (start, count); also the source-half split of
# the A contraction (S0 = 24 virtual tiles, S1 = 16).
HALVES = [(0, 6), (6, 4)]
K_ORDER = [r * NW + w
           for (w0, wn) in HALVES for r in range(N_CORES)
           for w in range(w0, w0 + wn)]
S_HALVES = [(0, 24), (24, 16)]
# dest chunk list per dest half (d0, dn); half 0 = windows 0-5 (768 dests),
# half 1 = windows 6-9 (512 dests)
DCH_H = [[(0, 512), (512, 256)], [(768, 512)]]
WINS_H = [range(0, 6), range(6, 10)]
# FC head node chunks
NCH = [(0, 512), (512, 512), (1024, 256)]


def _slot(k):
    """Global window index -> zsb slot (K_ORDER position)."""
    r, lw = divmod(k, NW)
    return r * 6 + lw if lw < 6 else 48 + r * 4 + (lw - 6)


_compiled_cache = {}
DEBUG = False


def build_nc():
    nc = bacc.Bacc("TRN2", target_bir_lowering=False, debug=False,
                   enable_asserts=True, num_devices=N_CORES)
    # ---------------- I/O ----------------
    xT_in = nc.dram_tensor("xT", [IN, NFULL], BF16, kind="ExternalInput")
    w_in = [nc.dram_tensor(f"w{k}", [MP, MP], BF16, kind="ExternalInput")
            for k in range(3)]
    b_in = [nc.dram_tensor(f"b{k}", [MP, 1], F32, kind="ExternalInput")
            for k in range(3)]
    i2_in = nc.dram_tensor("i2", [MP, MP], BF16, kind="ExternalInput")
    fcw0_in = nc.dram_tensor("fcw0", [MP, FL], BF16, kind="ExternalInput")
    fcw1_in = nc.dram_tensor("fcw1", [FL, FL], BF16, kind="ExternalInput")
    injw0_in = nc.dram_tensor("injw0", [MP, FL], BF16, kind="ExternalInput")
    injw1_in = nc.dram_tensor("injw1", [MP, FL], BF16, kind="ExternalInput")
    bh1_in = nc.dram_tensor("bh1", [FL, 1], F32, kind="ExternalInput")
    bh2_in = nc.dram_tensor("bh2", [FL, 1], F32, kind="ExternalInput")
    outw_in = nc.dram_tensor("outw", [FL, OUT], BF16, kind="ExternalInput")
    outb_in = nc.dram_tensor("outb", [OUT, 1], F32, kind="ExternalInput")
    dq_in = nc.dram_tensor("dq", [128, NK + 2 * NW], F32, kind="ExternalInput")
    invc_in = nc.dram_tensor("invc", [128, 4], F32, kind="ExternalInput")
    a_in = nc.dram_tensor("a_blk", [NVT, 128, 2 * NSH], FP8, kind="ExternalInput")
    out_t = nc.dram_tensor("outT", [OUT, NSH], F32, kind="ExternalOutput")
    if DEBUG:
        dbg_zsb1 = nc.dram_tensor("dbg_zsb1", [128, NK, MP], FP8,
                                  kind="ExternalOutput")
        dbg_zsb2 = nc.dram_tensor("dbg_zsb2", [128, NK, MP], FP8,
                                  kind="ExternalOutput")
        dbg_m2 = nc.dram_tensor("dbg_m2", [128, 2, NSH], BF16,
                                kind="ExternalOutput")
        dbg_f3 = nc.dram_tensor("dbg_f3", [128, 2, NSH], BF16,
                                kind="ExternalOutput")

    with tile.TileContext(nc) as tc:
        with tc.tile_pool(name="consts", bufs=1) as consts, \
             tc.tile_pool(name="work", bufs=1) as work, \
             tc.tile_pool(name="xp", bufs=3) as xp, \
             tc.tile_pool(name="dram", bufs=1, space="DRAM") as dram, \
             tc.tile_pool(name="ps_a", bufs=1, space="PSUM") as ps_a, \
             tc.tile_pool(name="ps_sm", bufs=3, space="PSUM") as ps_sm:

            # ---------------- collective warmup (first!) ----------------
            wu_in = dram.tile([128, 2], F32, name="wu_in", tag="wu_in")
            wu_out = dram.tile([128 * N_CORES, 2], F32, name="wu_out",
                               tag="wu_out", addr_space="Shared")
            wu_sb = work.tile([128, 2], F32, name="wu_sb", tag="wu_sb")
            nc.vector.memset(wu_sb[:], 0.0)
            nc.sync.dma_start(wu_in[:], wu_sb[:])
            nc.gpsimd.collective_compute(
                "AllGather", mybir.AluOpType.bypass,
                replica_groups=[list(range(N_CORES))],
                ins=[wu_in[:]], outs=[wu_out[:]])

            # ---------------- small constants ----------------
            w0_t = consts.tile([128, 2, MP], BF16, name="w0_t")
            nc.sync.dma_start(w0_t[:], w_in[0][:].rearrange("(k p) f -> p k f", p=128))
            w1_t = consts.tile([128, 2, MP], BF16, name="w1_t")
            nc.sync.dma_start(w1_t[:], w_in[1][:].rearrange("(k p) f -> p k f", p=128))
            b_t = []
            for k in range(3):
                bt = consts.tile([128, 2], F32, name=f"b_t{k}")
                nc.sync.dma_start(bt[:], b_in[k][:].rearrange("(k p) o -> p (k o)", p=128))
                b_t.append(bt)
            dq_t = consts.tile([128, NK + 2 * NW], F32, name="dq_t")
            nc.sync.dma_start(dq_t[:], dq_in[:])
            invc_t = consts.tile([128, 4], F32, name="invc_t")
            nc.sync.dma_start(invc_t[:], invc_in[:])
            w2_t = consts.tile([128, 2, MP], BF16, name="w2_t")
            nc.sync.dma_start(w2_t[:], w_in[2][:].rearrange("(k p) f -> p k f", p=128))
            i2_t = consts.tile([128, 2, MP], BF16, name="i2_t")
            nc.sync.dma_start(i2_t[:], i2_in[:].rearrange("(k p) f -> p k f", p=128))

            # ---------------- big persistent buffers ----------------
            a_res = consts.tile([128, NVT, 2, NSH], FP8, name="a_res")
            # double-buffered messages: pass li reads zsb[li % 2]; its
            # AllGather readback lands in zsb[(li+1) % 2] so the second
            # dest-half's matmuls still see the current layer's data.
            zsb = [consts.tile([128, NK, MP], FP8, name=f"zsb{i}")
                   for i in range(2)]
            z_nm = work.tile([128, NW, MP], FP8, name="z_nm")
            f3T = work.tile([128, 2, NSH], BF16, name="f3T")
            r3T = work.tile([128, 2, NSH], BF16, name="r3T")
            outT_sb = work.tile([OUT, NSH], F32, name="outT_sb")

            # ---------------- A passes ----------------
            def epilogue(li, hd, bank, mT):
                """li: 0/1/2 = pass consuming z1/z2/z3.  For li<2, produce
                m_{l+1} (feat-major bf16), the node-major z_{l+2} windows of
                this dest half, quantize, and fire the AllGather.  For li=2,
                produce f3/r3 for the FC head."""
                if li == 2:
                    for fh in range(2):
                        for ci, (d0, dn) in enumerate(DCH_H[hd]):
                            nc.scalar.activation(
                                r3T[:, fh, d0:d0 + dn], bank(hd, fh, ci),
                                RELU, scale=invc_t[:, 2:3])
                            nc.vector.tensor_tensor(
                                f3T[:, fh, d0:d0 + dn], bank(hd, fh, ci),
                                invc_t[:, 2:3].to_broadcast([128, dn]), op=MULT)
                    return
                # ci-outer so the dense matmuls on early dest chunks can
                # start after only two ACTs
                for ci, (d0, dn) in enumerate(DCH_H[hd]):
                    for fh in range(2):
                        nc.scalar.activation(
                            mT[:, fh, d0:d0 + dn], bank(hd, fh, ci),
                            RELU, bias=b_t[li + 1][:, fh:fh + 1],
                            scale=invc_t[:, li:li + 1])
                rhsw = w2_t if li == 0 else i2_t
                qcol0 = NK + li * NW  # 80 for li=0 (c2), 90 for li=1 (c3)
                wlist = list(WINS_H[hd])
                for w in wlist[::2]:            # window pairs
                    psn = ps_sm.tile([128, 512], F32, name="dps", tag="sm")
                    for wl in range(2):
                        for ki in range(2):
                            nc.tensor.matmul(
                                psn[:, wl * MP:(wl + 1) * MP],
                                mT[:, ki, (w + wl) * 128:(w + wl + 1) * 128],
                                rhsw[:, ki, :], start=(ki == 0), stop=(ki == 1))
                    nc.vector.tensor_tensor(
                        z_nm[:, w:w + 2, :],
                        psn[:].rearrange("p (w f) -> p w f", w=2),
                        dq_t[:, qcol0 + w:qcol0 + w + 2]
                        .to_broadcast([128, 2, MP]), op=MULT)
                w0_, wn = HALVES[hd]
                ag_in = dram.tile([128 * wn, MP], FP8, name=f"agi{li}{hd}",
                                  tag=f"agi{li}{hd}")
                ag_out = dram.tile([128 * wn * N_CORES, MP], FP8,
                                   name=f"ago{li}{hd}", tag=f"ago{li}{hd}",
                                   addr_space="Shared")
                nc.sync.dma_start(
                    ag_in[:].rearrange("(p w) f -> p w f", p=128),
                    z_nm[:, w0_:w0_ + wn, :])
                nc.gpsimd.collective_compute(
                    "AllGather", mybir.AluOpType.bypass,
                    replica_groups=[list(range(N_CORES))],
                    ins=[ag_in[:]], outs=[ag_out[:]])
                base = 0 if hd == 0 else 48
                znext = zsb[(li + 1) % 2]
                for r in range(N_CORES):
                    nc.sync.dma_start(
                        znext[:, base + r * wn:base + (r + 1) * wn, :],
                        ag_out[r * 128 * wn:(r + 1) * 128 * wn, :]
                        .rearrange("(p w) f -> p w f", p=128))

            def make_banks(li):
                b00 = ps_a.tile([128, 512], F32, name=f"b00_{li}", tag="b00")
                b01 = ps_a.tile([128, 512], F32, name=f"b01_{li}", tag="b01")
                bsh = ps_a.tile([128, 512], F32, name=f"bsh_{li}", tag="bsh")
                b10 = ps_a.tile([128, 512], F32, name=f"b10_{li}", tag="b10")
                b11 = ps_a.tile([128, 512], F32, name=f"b11_{li}", tag="b11")

                def bank(hd, fh, ci):
                    if hd == 0:
                        if ci == 0:
                            return b00[:] if fh == 0 else b01[:]
                        return bsh[:, fh * 256:(fh + 1) * 256]
                    return b10[:] if fh == 0 else b11[:]
                return bank

            def issue_vt(li, bank, vi, hds, first, last):
                """Issue the A matmuls of virtual tile vi for the dest
                halves in `hds`.  first/last flag the pass's first/last
                issued vt (psum group start/stop)."""
                zr = zsb[li % 2]
                for fh in range(2):
                    lhsT = zr[:, 2 * vi:2 * vi + 2, fh * 128:(fh + 1) * 128]
                    for hd in hds:
                        for ci, (d0, dn) in enumerate(DCH_H[hd]):
                            shared = (hd == 0 and ci == 1)
                            st = first and (fh == 0 or not shared)
                            sp = last and (fh == 1 or not shared)
                            nc.tensor.matmul(
                                bank(hd, fh, ci),
                                lhsT, a_res[:, vi, :, d0:d0 + dn],
                                start=st, stop=sp, perf_mode=DR,
                                skip_group_check=shared)

            def a_pass(li):
                """Passes 1 and 2 (li=1,2): dest-half pipelined."""
                bank = make_banks(li)
                mT = None
                if li < 2:
                    mT = work.tile([128, 2, NSH], BF16, name=f"mT{li}", tag="mT")
                for hd in range(2):
                    # bias the scheduler to finish dest-half 0 (and its
                    # epilogue/AllGather chain) before touching dest-half 1,
                    # so the collective fires as early as the data allows
                    import contextlib
                    prio = tc.high_priority(3000) if hd == 0 else \
                        contextlib.nullcontext()
                    with prio:
                        for (v0, vn) in S_HALVES:
                            for vi in range(v0, v0 + vn):
                                issue_vt(li, bank, vi, [hd],
                                         first=(vi == 0), last=(vi == NVT - 1))
                        epilogue(li, hd, bank, mT)
                return mT

            # ---------------- replicated layer-1 front ----------------
            # (the Tile framework's dependency scheduler overlaps pass 1's
            # matmuls with the front on its own — no manual interleave)
            for c in range(NXCH):
                xr = xp.tile([128, 2, 512], BF16, name="xr", tag="xr")
                nc.sync.dma_start(
                    xr[:], xT_in[:, c * 512:(c + 1) * 512]
                    .rearrange("(k p) n -> p k n", p=128))
                for v in (2 * c, 2 * c + 1):
                    nc.sync.dma_start(
                        a_res[:, v, :, :],
                        a_in[v, :, :].rearrange("p (k d) -> p k d", k=2))
                m1r = xp.tile([128, 2, 512], BF16, name="m1r", tag="m1r")
                for fo in range(2):
                    ps = ps_sm.tile([128, 512], F32, name="fps", tag="sm")
                    for ki in range(2):
                        nc.tensor.matmul(
                            ps[:], w0_t[:, ki, fo * 128:(fo + 1) * 128],
                            xr[:, ki, :], start=(ki == 0), stop=(ki == 1))
                    nc.scalar.activation(m1r[:, fo, :], ps[:], RELU,
                                         bias=b_t[0][:, fo:fo + 1])
                for wp in range(2):          # window pairs (2 per chunk)
                    w = 4 * c + 2 * wp
                    s = _slot(w)             # pair slots are adjacent
                    psn = ps_sm.tile([128, 512], F32, name="zps", tag="sm")
                    for wl in range(2):
                        for ki in range(2):
                            nc.tensor.matmul(
                                psn[:, wl * MP:(wl + 1) * MP],
                                m1r[:, ki, (2 * wp + wl) * 128:
                                    (2 * wp + wl + 1) * 128],
                                w1_t[:, ki, :],
                                start=(ki == 0), stop=(ki == 1))
                    nc.vector.tensor_tensor(
                        zsb[0][:, s:s + 2, :],
                        psn[:].rearrange("p (w f) -> p w f", w=2),
                        dq_t[:, s:s + 2].to_broadcast([128, 2, MP]),
                        op=MULT)

            # ---------------- FC head weights (loaded in background) -----
            fcw0_t = consts.tile([128, 2, FL], BF16, name="fcw0_t")
            nc.sync.dma_start(fcw0_t[:], fcw0_in[:].rearrange("(k p) f -> p k f", p=128))
            injw0_t = consts.tile([128, 2, FL], BF16, name="injw0_t")
            nc.sync.dma_start(injw0_t[:], injw0_in[:].rearrange("(k p) f -> p k f", p=128))
            injw1_t = consts.tile([128, 2, FL], BF16, name="injw1_t")
            nc.sync.dma_start(injw1_t[:], injw1_in[:].rearrange("(k p) f -> p k f", p=128))
            fcw1_t = consts.tile([128, 4, FL], BF16, name="fcw1_t")
            nc.sync.dma_start(fcw1_t[:], fcw1_in[:].rearrange("(k p) f -> p k f", p=128))
            outw_t = consts.tile([128, 4, OUT], BF16, name="outw_t")
            nc.sync.dma_start(outw_t[:], outw_in[:].rearrange("(k p) f -> p k f", p=128))
            bh1_t = consts.tile([128, 4], F32, name="bh1_t")
            nc.sync.dma_start(bh1_t[:], bh1_in[:].rearrange("(k p) o -> p (k o)", p=128))
            bh2_t = consts.tile([128, 4], F32, name="bh2_t")
            nc.sync.dma_start(bh2_t[:], bh2_in[:].rearrange("(k p) o -> p (k o)", p=128))
            outb_t = consts.tile([64, 1], F32, name="outb_t")
            nc.sync.dma_start(outb_t[:], outb_in[:])

            if DEBUG:
                nc.sync.dma_start(dbg_zsb1[:], zsb[0][:])
            m2T = a_pass(0)
            if DEBUG:
                nc.sync.dma_start(dbg_m2[:], m2T[:])
                nc.sync.dma_start(dbg_zsb2[:], zsb[1][:])
            a_pass(1)
            a_pass(2)
            if DEBUG:
                nc.sync.dma_start(dbg_f3[:], f3T[:])

            # ---------------- FC head (feat-major, chunked by nodes) -----
            for n0, nn in NCH:
                # h1 = relu(alpha*(r3 @ fc_w0) + f3 @ inj_w0 + bh1)
                r1 = work.tile([128, 4, FL], BF16, name="r1", tag="r1")
                for fo in range(4):
                    hp = ps_sm.tile([128, 512], F32, name="hp", tag="sm")
                    for ki in range(2):
                        nc.tensor.matmul(
                            hp[:, :nn], fcw0_t[:, ki, fo * 128:(fo + 1) * 128],
                            r3T[:, ki, n0:n0 + nn], start=(ki == 0), stop=False)
                    for ki in range(2):
                        nc.tensor.matmul(
                            hp[:, :nn], injw0_t[:, ki, fo * 128:(fo + 1) * 128],
                            f3T[:, ki, n0:n0 + nn], start=False, stop=(ki == 1))
                    nc.scalar.activation(r1[:, fo, :nn], hp[:, :nn], RELU,
                                         bias=bh1_t[:, fo:fo + 1])
                # h2 = alpha*(r1 @ fc_w1) + f3 @ inj_w1 + bh2  (no relu)
                h2 = work.tile([128, 4, FL], BF16, name="h2", tag="h2")
                for fo in range(4):
                    hp2 = ps_sm.tile([128, 512], F32, name="hp2", tag="sm")
                    for ki in range(4):
                        nc.tensor.matmul(
                            hp2[:, :nn], fcw1_t[:, ki, fo * 128:(fo + 1) * 128],
                            r1[:, ki, :nn], start=(ki == 0), stop=False)
                    for ki in range(2):
                        nc.tensor.matmul(
                            hp2[:, :nn], injw1_t[:, ki, fo * 128:(fo + 1) * 128],
                            f3T[:, ki, n0:n0 + nn], start=False, stop=(ki == 1))
                    nc.vector.tensor_tensor(
                        h2[:, fo, :nn], hp2[:, :nn],
                        bh2_t[:, fo:fo + 1].to_broadcast([128, nn]), op=ADD)
                # out = h2 @ out_w + out_b
                op_ = ps_sm.tile([64, 512], F32, name="op_", tag="sm")
                for ki in range(4):
                    nc.tensor.matmul(op_[:, :nn], outw_t[:, ki, :],
                                     h2[:, ki, :nn],
                                     start=(ki == 0), stop=(ki == 3))
                nc.vector.tensor_tensor(
                    outT_sb[:, n0:n0 + nn], op_[:, :nn],
                    outb_t[:].to_broadcast([64, nn]), op=ADD)

            nc.sync.dma_start(out_t[:], outT_sb[:])
    nc.compile()
    return nc


def _scatter_rows(row, col, h):
    """out[row] += h[col]; exact f32, used only for scale estimation."""
    try:
        import scipy.sparse as sp
        key = "_spA"
        A = _compiled_cache.get(key)
        if A is None:
            A = sp.coo_matrix((np.ones(E, np.float32), (row, col)),
                              shape=(N, N)).tocsr()
            _compiled_cache[key] = A
        return np.asarray(A @ h)
    except ImportError:
        out = np.zeros_like(h)
        np.add.at(out, row, h[col])
        return out


def _prep_inputs(x, edge_index, mp_w0, mp_b0, mp_w1, mp_b1, mp_w2, mp_b2,
                 fc_w0, fc_b0, fc_w1, fc_b1, inj_w0, inj_b0, inj_w1, inj_b1,
                 alpha, out_w, out_b):
    bf = ml_dtypes.bfloat16
    f8 = ml_dtypes.float8_e4m3
    x = np.asarray(x, dtype=np.float32)
    row = np.asarray(edge_index[0], dtype=np.int64)
    col = np.asarray(edge_index[1], dtype=np.int64)
    alpha = float(np.asarray(alpha))
    w0 = np.asarray(mp_w0, np.float32)
    w1 = np.asarray(mp_w1, np.float32)
    w2 = np.asarray(mp_w2, np.float32)
    b0 = np.asarray(mp_b0, np.float32)
    b1 = np.asarray(mp_b1, np.float32)
    b2 = np.asarray(mp_b2, np.float32)

    deg = np.bincount(col, minlength=N).astype(np.float32)
    deg_inv = 1.0 / np.maximum(deg, 1.0)

    # exact forward for per-layer fp8 scales
    m1 = np.maximum(x @ w0 + b0, 0.0)
    z1 = m1 @ w1
    h1 = z1 * deg_inv[:, None]
    c1 = SCALE_TGT / max(np.abs(h1).max(), 1e-30)
    m2 = np.maximum(_scatter_rows(row, col, h1) + b1, 0.0)
    z2 = m2 @ w2
    h2 = z2 * deg_inv[:, None]
    c2 = SCALE_TGT / max(np.abs(h2).max(), 1e-30)
    m3 = np.maximum(_scatter_rows(row, col, h2) + b2, 0.0)
    h3 = m3 * deg_inv[:, None]
    c3 = SCALE_TGT / max(np.abs(h3).max(), 1e-30)
    cs = [c1, c2, c3]

    # padded node layout
    xpad = np.zeros((NFULL, IN), dtype=np.float32)
    dinv_pad = np.zeros(NFULL, dtype=np.float32)
    for r in range(N_CORES):
        xpad[r * NSH:r * NSH + NSH_REAL] = x[r * NSH_REAL:(r + 1) * NSH_REAL]
        dinv_pad[r * NSH:r * NSH + NSH_REAL] = \
            deg_inv[r * NSH_REAL:(r + 1) * NSH_REAL]
    xT_bf = np.ascontiguousarray(xpad.T).astype(bf)

    # source -> (k, p): k = global window in K_ORDER slot space
    s_rank = col // NSH_REAL
    s_loc = col % NSH_REAL
    src_k = s_rank * NW + s_loc // 128
    src_p = s_loc % 128

    # dq: cols 0..79 = dinv*c1 per slot (replicated); 80..89 dinv*c2 own;
    # 90..99 dinv*c3 own
    dq_shared = np.zeros((128, NK + 2 * NW), dtype=np.float32)
    for s, k in enumerate(K_ORDER):
        dq_shared[:, s] = dinv_pad[k * 128:(k + 1) * 128] * c1

    invc = np.broadcast_to(
        np.array([1.0 / c1, 1.0 / c2, 1.0 / c3, 0.0], np.float32),
        (128, 4)).copy()

    shared = {
        "xT": xT_bf,
        "w0": w0.astype(bf), "w1": w1.astype(bf), "w2": w2.astype(bf),
        "b0": b0.reshape(MP, 1), "b1": b1.reshape(MP, 1),
        "b2": b2.reshape(MP, 1),
        "i2": np.eye(MP, dtype=np.float32).astype(bf),
        "fcw0": (alpha * np.asarray(fc_w0, np.float32)).astype(bf),
        "fcw1": (alpha * np.asarray(fc_w1, np.float32)).astype(bf),
        "injw0": np.asarray(inj_w0, np.float32).astype(bf),
        "injw1": np.asarray(inj_w1, np.float32).astype(bf),
        "bh1": (alpha * np.asarray(fc_b0, np.float32)
                + np.asarray(inj_b0, np.float32)).reshape(FL, 1),
        "bh2": (alpha * np.asarray(fc_b1, np.float32)
                + np.asarray(inj_b1, np.float32)).reshape(FL, 1),
        "outw": np.asarray(out_w, np.float32).astype(bf),
        "outb": np.asarray(out_b, np.float32).reshape(OUT, 1),
        "invc": invc,
    }

    in_maps = []
    korder = np.array(K_ORDER)
    for c in range(N_CORES):
        lo = c * NSH_REAL
        sel = (row >= lo) & (row < lo + NSH_REAL)
        d_local = (row[sel] - lo).astype(np.int64)
        a_blk = np.zeros((NK, 128, NSH), dtype=np.float32)
        np.add.at(a_blk, (src_k[sel], src_p[sel], d_local), 1.0)
        a_blk = a_blk[korder]
        a_blk = a_blk.reshape(NVT, 2, 128, NSH).transpose(0, 2, 1, 3) \
                     .reshape(NVT, 128, 2 * NSH)
        a_blk = np.ascontiguousarray(a_blk).astype(f8)

        dq = dq_shared.copy()
        for w in range(NW):
            k = c * NW + w
            dv = dinv_pad[k * 128:(k + 1) * 128]
            dq[:, NK + w] = dv * c2
            dq[:, NK + NW + w] = dv * c3

        m = dict(shared)
        m["dq"] = dq
        m["a_blk"] = a_blk
        in_maps.append(m)
    return in_maps


def kernel(**inputs):
    in_maps = _prep_inputs(**inputs)
    if "nc" not in _compiled_cache:
        _compiled_cache["nc"] = build_nc()
    nc = _compiled_cache["nc"]
    trace = _compiled_cache.get("trace", False)
    res = run_bass_kernel_spmd(nc, in_maps, core_ids=list(range(N_CORES)),
                               trace=trace)
    _compiled_cache["last_result"] = res
    out = np.zeros((N, OUT), dtype=np.float32)
    for c in range(N_CORES):
        out[c * NSH_REAL:(c + 1) * NSH_REAL, :] = \
            res.results[c]["outT"][:, :NSH_REAL].T
    return out
